# revision 40
# baseline (speedup 1.0000x reference)
"""Multi-head self-attention kernel for 8 Trainium2 NeuronCores.

Problem: B=2, S=2048, D=1024, H=16 heads, head_dim=64 (fp32 in/out).

Sharding: tensor-parallel over heads. Core c owns heads {2c, 2c+1}, i.e.
output-feature range [c*128, (c+1)*128) of the Q/K/V projections and the
matching 128 contraction rows of the output projection. Each core computes a
full-shape partial of the output; the host sums the 8 partials and adds bo.

Data path is bf16 (x, W, Q/K/V, attention probs, context, output partials
— all PSUM accumulation stays fp32; ~5e-3 final rel err), which halves DMA
and SBUF and keeps the PE at 1 cycle/row.

Per-core device program:
  1. QT/KT/VT [128, 4096] = W_shard @ x.T  (x.T pre-transposed on host).
  2. V' [k, kc, 65] per (batch, head): V plus a ones column (the softmax
     denominator falls out of the ctx matmul for free), built by PE
     transposes of VT into PSUM + one DVE evacuation per half.
  3. Attention runs per batch with BOTH heads in flight, outer loop over
     four 512-wide q-slices (PSUM: 2x2-bank scores tiles double-buffered +
     2 ctx accumulator banks + outproj/broadcast spares). Per k-chunk:
       scoresT[k=128, q=512] per head — head 0 contracts over PE rows 0-63,
         head 1 over rows 64-127 (tile_position auto-derives), so the two
         64-contraction matmuls share one PE pass (array tiling);
       PT = exp(0.125*scoresT) both heads in one ScalarE instr [128, 1024];
       ctx'T[65, 512] += V'_chunk.T @ PT per head (full-128 contraction),
         trailing the scores/exp stream by 3 chunks (software pipeline).
  4. Per q-slice: DVE reciprocals of the sum rows -> deferred division
     (emitted at chunk 2 of the next q-slice): K=1 PE outer products
     broadcast the reciprocals, DVE evacuates and scales ctx into CX,
     which unblocks four output t-chunks.
  5. out_partial[t=128, 1024] = CX_chunk.T @ WoT_shard per t-chunk,
     interleaved 1/chunk into the attention stream (own PSUM tag), PSUM
     evacuated on DVE (ACT for the last four, tail has ACT idle), bf16 DMA
     out. Host sums the 8 partials in fp32 and adds bo.
"""

import functools
import os
import sys

import numpy as np

for _p in ("/opt/trn_rl_repo", os.path.expanduser("~/.axon_site/_ro/trn_rl_repo")):
    if os.path.isdir(_p) and _p not in sys.path:
        sys.path.insert(0, _p)

import concourse.bass as bass
import concourse.tile as tile
from concourse import bacc
from concourse import mybir
from concourse.bass_utils import run_bass_kernel_spmd

F32 = mybir.dt.float32
F32R = mybir.dt.float32r
BF16 = mybir.dt.bfloat16
AF = mybir.ActivationFunctionType

P = 128          # partitions / feature slice per core
B = 2            # batch
S = 2048         # sequence length
D = 1024         # embed dim
T = B * S        # total tokens
HD = 64          # head dim
KO = D // P      # contraction subtiles for the projections
NT = 8           # t-tiles for the projections
TW = 512         # projection t-tile width / matmul free dim
NKC = S // P     # 128-wide k-chunks per (batch, head)
NQS = S // TW    # 512-wide q-slices per (batch, head)
N_CORES = 8
SCALE = 1.0 / np.sqrt(np.float32(HD))  # 0.125

# Direct SBUF->SBUF broadcast DMA (partition step 0 source). Fallback is a
# DRAM bounce, which is the known-good pattern from tile_groupnorm.
USE_SBUF_BCAST = False


def _build_nc(n_reps: int = 1, phases: str = "full", dyn_reps: bool = False):
    nc = bacc.Bacc(target_bir_lowering=False, debug=False, num_devices=N_CORES)

    if dyn_reps:
        reps = nc.declare_dram_parameter("reps", [1, 1], mybir.dt.int32, isOutput=False)
    # xt4[tt, ki, ko, t] = x[tt*TW + t, ko*P + ki]; per-partition-contiguous DMA
    xt4 = nc.declare_dram_parameter("xt4", [NT, P, KO, TW], BF16, isOutput=False)
    wqT = nc.declare_dram_parameter("wqT", [P, KO, P], BF16, isOutput=False)
    wkT = nc.declare_dram_parameter("wkT", [P, KO, P], BF16, isOutput=False)
    wvT = nc.declare_dram_parameter("wvT", [P, KO, P], BF16, isOutput=False)
    woT = nc.declare_dram_parameter("woT", [P, D], BF16, isOutput=False)
    bq = nc.declare_dram_parameter("bq", [P, 1], F32, isOutput=False)
    bk = nc.declare_dram_parameter("bk", [P, 1], F32, isOutput=False)
    bv = nc.declare_dram_parameter("bv", [P, 1], F32, isOutput=False)
    out = nc.declare_dram_parameter("out", [T, D], BF16, isOutput=True)

    with tile.TileContext(nc) as tc:
        from contextlib import ExitStack

        with ExitStack() as ctx:
            singles = ctx.enter_context(tc.tile_pool(name="singles", bufs=1))
            qkv = ctx.enter_context(tc.tile_pool(name="qkv", bufs=1))
            xpool = ctx.enter_context(tc.tile_pool(name="xpool", bufs=2))
            # 2*(CTX_LAG+1) pt tiles live at once (ctx trails scores by
            # CTX_LAG chunks, 2 halves per chunk)
            ptpool = ctx.enter_context(tc.tile_pool(name="ptpool", bufs=8))
            # 2 batches x 2 heads of V' live during the batch overlap
            vpool = ctx.enter_context(tc.tile_pool(name="vpool", bufs=4))
            # divisions never overlap across pairs (div N is consumed inside
            # pair N+1 before pair N+1's recips are emitted), so one buf
            rpool = ctx.enter_context(tc.tile_pool(name="rpool", bufs=1))
            opool = ctx.enter_context(tc.tile_pool(name="opool", bufs=3))
            sppool = ctx.enter_context(
                tc.tile_pool(name="sppool", bufs=2, space="PSUM")
            )
            pbank = ctx.enter_context(tc.tile_pool(name="pbank", bufs=3, space="PSUM"))
            if not USE_SBUF_BCAST:
                dpool = ctx.enter_context(
                    tc.tile_pool(name="dpool", bufs=2, space="DRAM")
                )
            pools = (singles, qkv, xpool, ptpool, vpool, rpool, opool, sppool,
                     pbank, dpool, xt4, wqT, wkT, wvT, woT, bq, bk, bv, out)

            from contextlib import nullcontext

            if dyn_reps:
                reps_sb = singles.tile([1, 1], mybir.dt.int32, name="reps_sb")
                nc.sync.dma_start(out=reps_sb[:], in_=reps[:])
                reps_val = nc.values_load(reps_sb[:], min_val=0, max_val=1 << 20)
                rep_loop = tc.For_i(0, reps_val, 1)
            elif n_reps > 1:
                rep_loop = tc.For_i(0, n_reps, 1)
            else:
                rep_loop = nullcontext()
            with rep_loop:
                _kernel_body(nc, tc, pools, phases)

    nc.finalize()
    return nc


def _kernel_body(nc, tc, pools, phases="full"):
    (singles, qkv, xpool, ptpool, vpool, rpool, opool, sppool, pbank, dpool,
     xt4, wqT, wkT, wvT, woT, bq, bk, bv, out) = pools
    if True:
        if True:
            # ---- weights / biases to SBUF ----
            w_sbs = []
            for name, wT in (("wq", wqT), ("wk", wkT), ("wv", wvT)):
                w_sb = singles.tile([P, KO, P], BF16, tag=f"{name}_sb")
                nc.sync.dma_start(out=w_sb[:], in_=wT[:])
                w_sbs.append(w_sb)
            # wo is first needed ~20 chunks in; load it off the critical
            # head DMA path (emitted after the first projection tile)
            wo_sb = singles.tile([P, D], BF16, tag="wo_sb")
            b_sbs = []
            for name, bdram in (("bq", bq), ("bk", bk), ("bv", bv)):
                b_sb = singles.tile([P, 1], F32, tag=f"{name}_sb")
                nc.sync.dma_start(out=b_sb[:], in_=bdram[:])
                b_sbs.append(b_sb)

            # ones row for PE-broadcast of softmax reciprocals (K=1 matmul)
            ones32 = singles.tile([1, HD], F32, tag="ones32")
            nc.vector.memset(ones32[:], 1.0)
            ones_r = singles.tile([1, HD], F32R, tag="ones_r")
            nc.vector.tensor_copy(ones_r[:], ones32[:])

            # identity for PE-transpose of V chunks (built on idle GpSimd)
            from concourse import masks as _masks

            # both partition halves hold I64 so either head's VT slice
            # (base partition 0 or 64) finds the identity at its own base
            ident32 = singles.tile([P, HD], BF16, tag="ident32")
            _masks.make_identity(nc, ident32[0:HD, :])
            _masks.make_identity(nc, ident32[HD:P, :])

            # ---- persistent activations ----
            QT = qkv.tile([P, T], BF16, tag="QT")
            KT = qkv.tile([P, T], BF16, tag="KT")
            VT = qkv.tile([P, T], BF16, tag="VT")
            CX = qkv.tile([P, T], BF16, tag="CX")  # scaled ctxT, both heads

            # ---- projections: QT/KT/VT[f, t] = sum_d W[d, f] * xT[d, t] ----

            def proj_tile(tt):
                xt = xpool.tile([P, KO, TW], BF16, tag="xt", name=f"xt_{tt}")
                # eight dma_starts spread across the HWDGE queues; finer
                # slices let the first ko matmul start sooner
                for q8 in range(KO):
                    nc.sync.dma_start(
                        out=xt[:, q8 : q8 + 1],
                        in_=xt4[:][tt, :, q8 : q8 + 1],
                    )
                for w_sb, b_sb, dst in zip(w_sbs, b_sbs, (QT, KT, VT), strict=True):
                    ps = sppool.tile([P, TW], F32, tag="sp")
                    for ko in range(KO):
                        nc.tensor.matmul(
                            ps[:],
                            w_sb[:, ko],
                            xt[:, ko],
                            start=(ko == 0),
                            stop=(ko == KO - 1),
                        )
                    nc.vector.tensor_scalar_add(
                        dst[:, tt * TW : (tt + 1) * TW], ps[:], b_sb[:]
                    )
                    yield

            # only projection tile 0 upfront: attention q-slice 0 needs
            # just Q/K of tokens 0-511 to start; the other seven tiles
            # interleave into the attention stream
            for _ in proj_tile(0):
                pass
            nc.sync.dma_start(out=wo_sb[:], in_=woT[:])

            # ---- attention for one (batch, head) pair (generator:
            #      yields after V' build and after each k-chunk) ----
            def vbuild_alloc(bb: int, h: int):
                # V' [k-part, kc, 65]: V plus a ones column (the 65th). The
                # tile is allocated (and its ones column set) up front; the
                # V data is filled per half by vbuild_fill once the V
                # projection for those tokens has been emitted.
                vp = vpool.tile(
                    [P, NKC, HD + 1], BF16, tag="vp", name=f"vp_{bb}_{h}"
                )
                nc.vector.memset(vp[:, :, HD], 1.0)
                return vp

            def vbuild_fill(bb: int, h: int, half: int, vp):
                # PE transposes of VT 128-col chunks into PSUM (~40ns each),
                # one DVE convert-copy per half to evacuate.
                base = bb * S
                pb = h * HD
                NH = NKC // 2
                tp = sppool.tile(
                    [P, NH * HD], BF16, tag="sp", name=f"vtp_{bb}_{h}_{half}"
                )
                for j in range(NH):
                    kc = half * NH + j
                    nc.tensor.matmul(
                        tp[:, j * HD : (j + 1) * HD],
                        VT[pb : pb + HD, base + kc * P : base + (kc + 1) * P],
                        ident32[pb : pb + HD, :],
                        is_transpose=True,
                    )
                nc.vector.tensor_copy(
                    vp[:, half * NH : (half + 1) * NH, 0:HD],
                    tp[:].rearrange("p (kc d) -> p kc d", d=HD),
                )

            def vbuild(bb: int, h: int):
                vp = vbuild_alloc(bb, h)
                vbuild_fill(bb, h, 0, vp)
                vbuild_fill(bb, h, 1, vp)
                return vp

            def _make_div(bb, q4, ctx_pair, rs_r, pending):
                # deferred: broadcast both heads' reciprocals (col-tiled K=1
                # matmuls), evacuate once, scale CX rows of both heads, then
                # release the four output t-chunks this q-slice unblocks.
                def division():
                    base = bb * S
                    # both broadcasts at dst base partition 0 (the ISA
                    # rejects K=1 matmuls landing at partition 64), heads
                    # side by side in columns; the copies split them into
                    # rb's partition halves
                    rbp = sppool.tile(
                        [HD, 2 * TW], F32, tag="sp", name=f"rbp_{bb}_{q4}"
                    )
                    for h in range(2):
                        nc.tensor.matmul(
                            rbp[:, h * TW : (h + 1) * TW],
                            ones_r[:],
                            rs_r[:, h * TW : (h + 1) * TW],
                            start=True,
                            stop=True,
                        )
                    rb = rpool.tile([P, TW], F32, tag="rb", name=f"rb_{bb}_{q4}")
                    for h in range(2):
                        nc.vector.tensor_copy(
                            rb[h * HD : (h + 1) * HD, :],
                            rbp[:, h * TW : (h + 1) * TW],
                        )
                    for h in range(2):
                        pb = h * HD
                        nc.vector.tensor_mul(
                            out=CX[
                                pb : pb + HD,
                                base + q4 * TW : base + (q4 + 1) * TW,
                            ],
                            in0=ctx_pair[h][0:HD, :],
                            in1=rb[pb : pb + HD, :],
                        )
                    pending[0] += 4
                return division

            def batch_attn(bb: int, vps, prev_div, opj, pending):
                # Both heads of batch bb concurrently via PE array row
                # tiling: head 0's scores matmul contracts over rows 0-63,
                # head 1's over rows 64-127 (tile_position auto-derives from
                # the operands' base partitions), so the two 64-contraction
                # matmuls share one PE pass. Outer loop over four 512-wide
                # q-slices keeps PSUM inside 8 banks: scores 2x2 banks
                # double-buffered + 2x2 ctx accumulator banks.
                base = bb * S
                CTX_LAG = 3
                yield
                pend_div = prev_div
                ctx_live = {}
                pts = {}

                def emit_scores(q4, kc):
                    qb = base + q4 * TW
                    sp = sppool.tile(
                        [P, 2 * TW], F32, tag="sp", name=f"sp_{bb}_{q4}_{kc}"
                    )
                    for h in range(2):
                        pb = h * HD
                        nc.tensor.matmul(
                            sp[:, h * TW : (h + 1) * TW],
                            KT[pb : pb + HD, base + kc * P : base + (kc + 1) * P],
                            QT[pb : pb + HD, qb : qb + TW],
                            start=True,
                            stop=True,
                        )
                    pt = ptpool.tile(
                        [P, 2 * TW], BF16, tag="pt", name=f"pt_{bb}_{q4}_{kc}"
                    )
                    nc.scalar.activation(pt[:], sp[:], AF.Exp, scale=float(SCALE))
                    pts[(q4, kc)] = pt

                def emit_ctx(q4, kc):
                    if q4 not in ctx_live:
                        ctx_live[q4] = [
                            pbank.tile(
                                [HD + 1, TW], F32, tag="pb",
                                name=f"ctx_{bb}_{q4}_{h}",
                            )
                            for h in range(2)
                        ]
                    pt = pts.pop((q4, kc))
                    for h in range(2):
                        nc.tensor.matmul(
                            ctx_live[q4][h][:],
                            vps[h][:, kc],
                            pt[:, h * TW : (h + 1) * TW],
                            start=(kc == 0),
                            stop=(kc == NKC - 1),
                        )

                for q4 in range(4):
                    for kc in range(NKC):
                        emit_scores(q4, kc)
                        if kc == 2 and pend_div is not None:
                            pend_div()
                            pend_div = None
                        if kc >= CTX_LAG:
                            emit_ctx(q4, kc - CTX_LAG)
                        if pending[0] > 0:
                            pending[0] -= 1
                            next(opj, None)
                        yield
                    for kc in range(NKC - CTX_LAG, NKC):
                        emit_ctx(q4, kc)
                    ctx_pair = ctx_live.pop(q4)
                    # both heads' softmax denominators -> reciprocals, eagerly
                    rs = rpool.tile(
                        [1, 2 * TW], F32, tag="rs", name=f"rs_{bb}_{q4}"
                    )
                    rs_r = rpool.tile(
                        [1, 2 * TW], F32R, tag="rs_r", name=f"rsr_{bb}_{q4}"
                    )
                    for h in range(2):
                        nc.vector.reciprocal(
                            rs[:, h * TW : (h + 1) * TW],
                            ctx_pair[h][HD : HD + 1, :],
                        )
                        nc.vector.tensor_copy(
                            rs_r[:, h * TW : (h + 1) * TW],
                            rs[:, h * TW : (h + 1) * TW],
                        )
                    pend_div = _make_div(bb, q4, ctx_pair, rs_r, pending)

                yield pend_div

            # ---- output projection for one batch (generator) ----
            def outproj(bb: int):
                # PSUM evacuations on DVE while interleaved into attention
                # (ACT is the bottleneck there); the last four t-chunks run
                # in the tail with ACT idle, so split across both engines.
                for tci in range(S // P):
                    tg = bb * (S // P) + tci
                    act_evac = bb == 1 and tci >= 12
                    ot = opool.tile([P, D], BF16, tag="ot")
                    for half in range(2):
                        ps = sppool.tile([P, TW], F32, tag="op_ps", bufs=1)
                        nc.tensor.matmul(
                            ps[:],
                            CX[:, tg * P : (tg + 1) * P],
                            wo_sb[:, half * TW : (half + 1) * TW],
                            start=True,
                            stop=True,
                        )
                        if half == 1 and act_evac:
                            nc.scalar.copy(
                                ot[:, half * TW : (half + 1) * TW], ps[:]
                            )
                        else:
                            nc.vector.tensor_copy(
                                ot[:, half * TW : (half + 1) * TW], ps[:]
                            )
                        nc.sync.dma_start(
                            out=out[:][
                                tg * P : (tg + 1) * P, half * TW : (half + 1) * TW
                            ],
                            in_=ot[:, half * TW : (half + 1) * TW],
                        )
                    yield

            if phases == "proj":
                for tt in range(NT // 2, NT):
                    for _ in proj_tile(tt):
                        pass
                for i, t_ in enumerate((QT, KT, VT)):
                    for j in range(4):
                        nc.sync.dma_start(
                            out=out[:][(4 * i + j) * P : (4 * i + j + 1) * P, :],
                            in_=t_[:, j * D : (j + 1) * D],
                        )
                return

            import itertools

            def run_pair(gen, prev_div=None, interleave=None, per_step=0,
                         interleave_from=1, actions=None):
                # Drive a pair generator. Yields are: V'-build, one per
                # k-chunk, then the pair's deferred division closure. The
                # previous pair's division is emitted right after this pair's
                # first chunk (so its PE work never blocks the boundary), and
                # any interleaved work (projections / outproj) starts only
                # after that, preserving emission-order dependencies.
                division = None
                n = 0
                for item in gen:
                    if callable(item):
                        division = item
                        continue
                    n += 1
                    # n==3: after two scores chunks, so the division's PE
                    # broadcast (which waits on DVE recips) never heads the
                    # PE queue at a pair boundary. Must still be before the
                    # first emit_ctx (n==CTX_LAG+2) which reallocates the
                    # PSUM banks the division reads.
                    if prev_div is not None and n == 3:
                        prev_div()
                        prev_div = None
                    if interleave is not None and n >= interleave_from:
                        for _ in range(per_step):
                            next(interleave, None)
                    if actions and n in actions:
                        actions[n]()
                if prev_div is not None:
                    prev_div()
                return division

            # batch 0 attention interleaved with batch-1 projections.
            # V' tiles for batch 1 are built mid-batch-0 (action at chunk
            # 34, after the interleaved projections finish); each q-slice's
            # division is deferred into the following q-slice's chunk 2 and
            # releases four output t-chunks.
            proj_steps = itertools.chain(
                *[proj_tile(tt) for tt in range(1, NT)]
            )
            vps0 = [vbuild_alloc(0, 0), vbuild_alloc(0, 1)]
            vps1 = []

            def fill0_half(half):
                def act():
                    vbuild_fill(0, 0, half, vps0[0])
                    vbuild_fill(0, 1, half, vps0[1])
                return act

            def build_vps1():
                vps1.append(vbuild(1, 0))
                vps1.append(vbuild(1, 1))

            op_chain = itertools.chain(outproj(0), outproj(1))
            op_pending = [0]
            div03 = run_pair(
                batch_attn(0, vps0, None, op_chain, op_pending), None,
                proj_steps, per_step=2, interleave_from=1,
                actions={3: fill0_half(0), 8: fill0_half(1), 34: build_vps1},
            )
            for _ in proj_steps:
                pass
            div13 = run_pair(batch_attn(1, vps1, div03, op_chain, op_pending))
            div13()
            for _ in op_chain:
                pass


@functools.lru_cache(maxsize=8)
def _get_nc(n_reps: int = 1, phases: str = "full", dyn_reps: bool = False):
    return _build_nc(n_reps, phases, dyn_reps)


def _shard_inputs(x, Wq, bq, Wk, bk, Wv, bv, Wo, bo):
    import ml_dtypes

    bf16 = ml_dtypes.bfloat16
    x = np.asarray(x, dtype=np.float32).astype(bf16)
    # xt4[tt, ki, ko, t] = x[tt*TW + t, ko*P + ki]
    xt4 = np.ascontiguousarray(
        x.reshape(NT, TW, KO, P).transpose(0, 3, 2, 1)
    )
    Wq = np.asarray(Wq, dtype=np.float32).astype(bf16)
    Wk = np.asarray(Wk, dtype=np.float32).astype(bf16)
    Wv = np.asarray(Wv, dtype=np.float32).astype(bf16)
    Wo = np.asarray(Wo, dtype=np.float32).astype(bf16)
    bq = np.asarray(bq, dtype=np.float32)
    bk = np.asarray(bk, dtype=np.float32)
    bv = np.asarray(bv, dtype=np.float32)

    def wtile(W, sl):
        # [ki, ko, f] = W[c*P + f, ko*P + ki]
        return np.ascontiguousarray(
            W[sl, :].reshape(P, KO, P).transpose(2, 1, 0)
        )

    in_maps = []
    for c in range(N_CORES):
        sl = slice(c * P, (c + 1) * P)
        in_maps.append(
            {
                "xt4": xt4,
                "wqT": wtile(Wq, sl),
                "wkT": wtile(Wk, sl),
                "wvT": wtile(Wv, sl),
                "woT": np.ascontiguousarray(Wo[:, sl].T),
                "bq": np.ascontiguousarray(bq[sl].reshape(P, 1)),
                "bk": np.ascontiguousarray(bk[sl].reshape(P, 1)),
                "bv": np.ascontiguousarray(bv[sl].reshape(P, 1)),
            }
        )
    return in_maps


def kernel(x, Wq, bq, Wk, bk, Wv, bv, Wo, bo, **run_kwargs):
    nc = _get_nc()
    in_maps = _shard_inputs(x, Wq, bq, Wk, bk, Wv, bv, Wo, bo)
    last_exc = None
    for _attempt in range(3):
        try:
            res = run_bass_kernel_spmd(
                nc, in_maps, core_ids=list(range(N_CORES)), **run_kwargs
            )
            break
        except Exception as exc:  # transient device errors: retry
            last_exc = exc
            import time as _time

            _time.sleep(3.0)
            # a wedged PJRT client never recovers in-process; force a fresh
            # backend connection so the retry sees recovered devices
            try:
                import jax as _jax

                _jax.clear_caches()
                _jax.extend.backend.clear_backends()
            except Exception:
                pass
    else:
        raise last_exc
    partials = [r["out"] for r in res.results]
    acc = np.add.reduce([np.asarray(p, dtype=np.float32) for p in partials], axis=0)
    acc = acc + np.asarray(bo, dtype=np.float32)[None, :]
    if run_kwargs:
        kernel.last_results = res
    return acc.reshape(B, S, D).astype(np.float32)

